# revision 6
# baseline (speedup 1.0000x reference)
"""Trainium2 Bass kernel for nn_GSDRWLSNet (gnn_message_passing).

Reference computation (N=1024, J=4, BLOCK=512, ALPHA=0.1):
  feats = x @ Wb; xb_j = feats block j
  S[j,a,b,k] = xb_j[a] W_jk xb_j[b] + b_jk
  A = row-softmax(S with diag masked) over b       [J,2,N,N] transition mats
  A_an = (1-a) (I - a A)^-1
  one[j,i] = a S_j A_ik^T + (1-a) S_j ; ana[j,i] = S_j A_an_ik^T
  out = concat over (one, ana) rows -> [J*J*2*N*N, 2]

Sharding: 8 cores = (i, k) pairs.  Core (i,k) computes its masked softmax +
(I - aA)^-1 (5-term Neumann/Horner series; alpha=0.1 so truncation error
~1e-6, far below fp32r matmul noise) + all J left-products for its (i,k).
SPMD: the same program runs on every core; per-core behavior comes only
from the data (the j-blocks of Wb/W_embed are rotated per core so block 0
is always that core's i-block).

All matmuls run in float32r (~1.5e-4 rel err, 4x the throughput of fp32
on the PE array).  S_j^T matrices stage through an HBM scratch tile to
stay inside SBUF; S_0's row layout is recovered from that scratch with PE
transposes so the big featsT/T^T tiles can be freed first (SBUF pools are
stack-allocated).
"""

import numpy as np

ALPHA = 0.1
N = 1024
J = 4
B = 512
T_HORNER = 5  # Neumann terms through q^5

_CACHE = {}


def _build_module():
    """Build + compile the single-program SPMD Bass module."""
    if "nc" in _CACHE:
        return _CACHE["nc"]

    import concourse.tile as tile
    import concourse.mybir as mybir
    from concourse import bacc
    from concourse.masks import make_identity

    dt = mybir.dt
    FPR = dt.float32r
    F32 = dt.float32
    Alu = mybir.AluOpType
    Act = mybir.ActivationFunctionType
    AX = mybir.AxisListType

    nc = bacc.Bacc("TRN2", target_bir_lowering=False, debug=False, num_devices=8)

    xT = nc.dram_tensor("xT", [N, N], FPR, kind="ExternalInput")
    Wb = nc.dram_tensor("Wb", [N, 2 * N], FPR, kind="ExternalInput")
    Wk = nc.dram_tensor("Wk", [J * B, B], FPR, kind="ExternalInput")
    bk = nc.dram_tensor("bk", [128, J], F32, kind="ExternalInput")
    out = nc.dram_tensor("out", [J, 2, N, N], F32, kind="ExternalOutput")

    C = 1024  # column width of one 128-row chunk in flat [128, n*1024] tiles

    with tile.TileContext(nc) as tc:
        consts = tc.alloc_tile_pool(name="consts", bufs=1)
        psum_pool = tc.alloc_tile_pool(name="psum", bufs=8, space="PSUM")
        stage_pool = tc.alloc_tile_pool(name="stage", bufs=4)
        ostage_pool = tc.alloc_tile_pool(name="ostage", bufs=3)
        dram_pool = tc.alloc_tile_pool(name="sTdp", bufs=1, space="DRAM")

        # ---- constants ----
        i128f = consts.tile([128, 128], F32)
        make_identity(nc, i128f[:])
        maskd = consts.tile([128, 128], F32)  # 1 - I
        nc.vector.tensor_scalar(
            out=maskd[:], in0=i128f[:], scalar1=-1.0, scalar2=1.0,
            op0=Alu.mult, op1=Alu.add)
        i128r = consts.tile([128, 128], FPR)
        nc.vector.tensor_copy(i128r[:], i128f[:])
        i09r = consts.tile([128, 128], FPR)  # (1-ALPHA) * I
        nc.vector.tensor_scalar(
            out=i09r[:], in0=i128f[:], scalar1=1.0 - ALPHA, scalar2=None,
            op0=Alu.mult)
        strips = []
        for t in range(4):
            s = consts.tile([128, 512], FPR, tag=f"strip{t}")
            nc.gpsimd.memset(s[:].bitcast(F32), 0.0)
            nc.vector.tensor_copy(s[:, t * 128:(t + 1) * 128], i128f[:])
            strips.append(s)
        bkb = consts.tile([128, J], F32)
        nc.sync.dma_start(bkb[:], bk.ap()[:])

        sTd = dram_pool.tile([J * N, N], FPR)  # S_j^T scratch in HBM

        def products(j, buf, rhs, part, scale):
            """out[j, part] = scale * (S_j @ rhs); buf holds S_j^T."""
            for mt in range(8):  # a row-chunks
                ot = ostage_pool.tile([128, C], F32, tag="ot")
                for nt in range(2):  # c columns
                    p = psum_pool.tile([128, 512], F32, tag="ps")
                    for kt in range(8):  # b contraction
                        nc.tensor.matmul(
                            p[:],
                            buf[:, kt * C + mt * 128:kt * C + mt * 128 + 128],
                            rhs[:, kt * C + nt * 512:kt * C + nt * 512 + 512],
                            start=(kt == 0), stop=(kt == 7))
                    osl = slice(nt * 512, nt * 512 + 512)
                    if scale is None:
                        nc.vector.tensor_copy(ot[:, osl], p[:])
                    else:
                        nc.vector.tensor_scalar(
                            out=ot[:, osl], in0=p[:], scalar1=scale,
                            scalar2=None, op0=Alu.mult)
                nc.sync.dma_start(
                    out.ap()[j, part, mt * 128:mt * 128 + 128, :], ot[:])

        # ---- phase A: featsT = Wb^T @ x^T ----
        p_feats = tc.alloc_tile_pool(name="p_feats", bufs=1)
        featsT = p_feats.tile([128, 16 * C], FPR)
        p_io1 = tc.alloc_tile_pool(name="p_io1", bufs=1)
        xTt = p_io1.tile([128, 8 * C], FPR)
        for t in range(8):
            nc.sync.dma_start(
                xTt[:, t * C:(t + 1) * C], xT.ap()[t * 128:(t + 1) * 128, :])
        for half in range(2):
            Wbt = p_io1.tile([128, 8 * C], FPR, tag="wbh")
            for t in range(8):
                nc.sync.dma_start(
                    Wbt[:, t * C:(t + 1) * C],
                    Wb.ap()[t * 128:(t + 1) * 128,
                            half * 1024:(half + 1) * 1024])
            for fl in range(8):  # f tile within this half
                ft = half * 8 + fl
                for nt in range(2):
                    p = psum_pool.tile([128, 512], F32, tag="ps")
                    for kt in range(8):
                        nc.tensor.matmul(
                            p[:],
                            Wbt[:, kt * C + fl * 128:kt * C + fl * 128 + 128],
                            xTt[:, kt * C + nt * 512:kt * C + nt * 512 + 512],
                            start=(kt == 0), stop=(kt == 7))
                    nc.vector.tensor_copy(
                        featsT[:, ft * C + nt * 512:ft * C + nt * 512 + 512],
                        p[:])
        p_io1.release()

        # ---- phase B: T_j^T = W_jk^T X_j^T ----
        p_tT = tc.alloc_tile_pool(name="p_tT", bufs=1)
        tTt = p_tT.tile([128, 16 * C], FPR)
        p_wk = tc.alloc_tile_pool(name="p_wk", bufs=1)
        Wkt = p_wk.tile([128, 16 * 512], FPR)
        for t in range(16):
            nc.sync.dma_start(
                Wkt[:, t * 512:(t + 1) * 512], Wk.ap()[t * 128:(t + 1) * 128, :])
        for j in range(J):
            for mt in range(4):
                for nt in range(2):
                    p = psum_pool.tile([128, 512], F32, tag="ps")
                    for kt in range(4):
                        ch = 4 * j + kt
                        nc.tensor.matmul(
                            p[:],
                            Wkt[:, ch * 512 + mt * 128:ch * 512 + mt * 128 + 128],
                            featsT[:, ch * C + nt * 512:ch * C + nt * 512 + 512],
                            start=(kt == 0), stop=(kt == 3))
                    nc.vector.tensor_copy(
                        tTt[:, (4 * j + mt) * C + nt * 512:
                            (4 * j + mt) * C + nt * 512 + 512], p[:])
        p_wk.release()

        # ---- phase C: S_j^T (+bias) -> HBM scratch ----
        for j in range(J):
            for mt in range(8):  # b row-chunks
                for nt in range(2):  # a columns
                    p = psum_pool.tile([128, 512], F32, tag="ps")
                    for kt in range(4):  # c contraction
                        ch = 4 * j + kt
                        nc.tensor.matmul(
                            p[:],
                            featsT[:, ch * C + mt * 128:ch * C + mt * 128 + 128],
                            tTt[:, ch * C + nt * 512:ch * C + nt * 512 + 512],
                            start=(kt == 0), stop=(kt == 3))
                    st = stage_pool.tile([128, 512], FPR, tag="st")
                    nc.vector.tensor_scalar(
                        out=st[:], in0=p[:], scalar1=bkb[:, j:j + 1],
                        scalar2=None, op0=Alu.add)
                    nc.sync.dma_start(
                        sTd[j * N + mt * 128:j * N + mt * 128 + 128,
                            nt * 512:nt * 512 + 512], st[:])
        p_tT.release()
        p_feats.release()

        # ---- phase D: eh = exp(S_0) via PE-transpose of S_0^T ----
        p_Pa = tc.alloc_tile_pool(name="p_Pa", bufs=1)
        p_aA = tc.alloc_tile_pool(name="p_aA", bufs=1)
        p_stats = tc.alloc_tile_pool(name="p_stats", bufs=1)
        p_eh = tc.alloc_tile_pool(name="p_eh", bufs=1)
        eh = p_eh.tile([128, 8 * C], F32)
        aA = p_aA.tile([128, 8 * C], FPR)
        zsum = p_stats.tile([128, 8], F32)
        zr2 = p_stats.tile([128, 8], F32)

        p_s0 = tc.alloc_tile_pool(name="p_s0", bufs=1)
        s0buf = p_s0.tile([128, 8 * C], FPR)
        s0row = p_s0.tile([128, 8 * C], F32)
        for bt in range(8):
            nc.sync.dma_start(
                s0buf[:, bt * C:(bt + 1) * C],
                sTd[bt * 128:bt * 128 + 128, :])
        for ra in range(8):  # a row-chunks of S_0
            for bt in range(8):  # b columns
                p = psum_pool.tile([128, 512], FPR, tag="ps")
                nc.tensor.transpose(
                    p[:, 0:128],
                    s0buf[:, bt * C + ra * 128:bt * C + ra * 128 + 128],
                    i128r[:])
                nc.vector.tensor_copy(
                    s0row[:, ra * C + bt * 128:ra * C + bt * 128 + 128],
                    p[:, 0:128])
        for ra in range(8):
            nc.scalar.activation(
                eh[:, ra * C:(ra + 1) * C], s0row[:, ra * C:(ra + 1) * C],
                Act.Exp)
        p_s0.release()

        # ---- phase E: masked softmax -> aA = alpha*A ----
        for ra in range(8):
            blk = eh[:, ra * C + ra * 128:ra * C + ra * 128 + 128]
            nc.vector.tensor_mul(blk, blk, maskd[:])
        for ra in range(8):
            nc.vector.reduce_sum(
                zsum[:, ra:ra + 1], eh[:, ra * C:(ra + 1) * C], axis=AX.X)
        nc.vector.reciprocal(zr2[:], zsum[:])
        nc.vector.tensor_scalar(
            out=zr2[:], in0=zr2[:], scalar1=ALPHA, scalar2=None, op0=Alu.mult)
        for ra in range(8):
            nc.vector.tensor_scalar(
                out=aA[:, ra * C:(ra + 1) * C],
                in0=eh[:, ra * C:(ra + 1) * C],
                scalar1=zr2[:, ra:ra + 1], scalar2=None, op0=Alu.mult)
        p_eh.release()

        # ---- phase F: q = (alpha A)^T via identity strips;
        #      qp = q + (1-a) I (for `one`),  Pa = q + I ----
        p_qp = tc.alloc_tile_pool(name="p_qp", bufs=1)
        p_Pb = tc.alloc_tile_pool(name="p_Pb", bufs=1)
        qp = p_qp.tile([128, 8 * C], FPR)
        Pa = p_Pa.tile([128, 8 * C], FPR)
        Pb = p_Pb.tile([128, 8 * C], FPR)
        for bt in range(8):
            for nt in range(2):
                p = psum_pool.tile([128, 512], F32, tag="ps")
                for t in range(4):
                    kt = nt * 4 + t
                    nc.tensor.matmul(
                        p[:],
                        aA[:, kt * C + bt * 128:kt * C + bt * 128 + 128],
                        strips[t][:],
                        start=(t == 0), stop=(t == 3))
                sl = slice(bt * C + nt * 512, bt * C + nt * 512 + 512)
                nc.vector.tensor_copy(qp[:, sl], p[:])
                nc.vector.tensor_copy(Pa[:, sl], p[:])
        for bt in range(8):
            dsl = slice(bt * C + bt * 128, bt * C + bt * 128 + 128)
            nc.vector.tensor_add(qp[:, dsl], qp[:, dsl], i09r[:])
            nc.vector.tensor_add(Pa[:, dsl], Pa[:, dsl], i128r[:])

        # ---- phase G: one_j interleaved with Horner steps ----
        def horner_step(src, dst):
            for bt in range(8):
                for nt in range(2):
                    p = psum_pool.tile([128, 512], F32, tag="ps")
                    for kt in range(8):
                        nc.tensor.matmul(
                            p[:],
                            aA[:, kt * C + bt * 128:kt * C + bt * 128 + 128],
                            src[:, kt * C + nt * 512:kt * C + nt * 512 + 512],
                            start=(kt == 0), stop=(kt == 7))
                    nc.vector.tensor_copy(
                        dst[:, bt * C + nt * 512:bt * C + nt * 512 + 512], p[:])
            for bt in range(8):
                dsl = slice(bt * C + bt * 128, bt * C + bt * 128 + 128)
                nc.vector.tensor_add(dst[:, dsl], dst[:, dsl], i128r[:])

        hsrc, hdst = Pa, Pb
        p_ob = tc.alloc_tile_pool(name="p_ob", bufs=1)
        for j in range(J):
            buf = p_ob.tile([128, 8 * C], FPR, tag="onebuf")
            for bt in range(8):
                nc.sync.dma_start(
                    buf[:, bt * C:(bt + 1) * C],
                    sTd[j * N + bt * 128:j * N + bt * 128 + 128, :])
            products(j, buf, qp, 0, None)
            if j < T_HORNER - 1:
                horner_step(hsrc, hdst)
                hsrc, hdst = hdst, hsrc
        p_ob.release()
        assert hsrc is Pa, "T_HORNER=5 must end in Pa"
        p_Pb.release()
        p_qp.release()
        p_stats.release()
        p_aA.release()

        # ---- phase H: ana_j = (1-a) S_j @ P ----
        p_ab = tc.alloc_tile_pool(name="p_ab", bufs=2)
        for j in range(J):
            buf = p_ab.tile([128, 8 * C], FPR, tag="anabuf")
            for bt in range(8):
                nc.sync.dma_start(
                    buf[:, bt * C:(bt + 1) * C],
                    sTd[j * N + bt * 128:j * N + bt * 128 + 128, :])
            products(j, buf, Pa, 1, 1.0 - ALPHA)
        p_ab.release()
        p_Pa.release()

        ostage_pool.release()
        stage_pool.release()
        dram_pool.release()
        psum_pool.release()
        consts.release()

    nc.compile()
    _CACHE["nc"] = nc
    return nc


def _in_maps(x, Wb, W_embed, b_embed):
    x = np.ascontiguousarray(np.asarray(x, dtype=np.float32))
    Wb = np.ascontiguousarray(np.asarray(Wb, dtype=np.float32))
    W_embed = np.asarray(W_embed, dtype=np.float32)
    b_embed = np.asarray(b_embed, dtype=np.float32)

    xT = np.ascontiguousarray(x.T)
    in_maps, orders = [], []
    for c in range(8):
        i, k = c % 4, c // 4
        order = [(i + m) % 4 for m in range(4)]
        orders.append(order)
        Wb_p = np.ascontiguousarray(
            np.concatenate([Wb[:, B * j:B * (j + 1)] for j in order], axis=1))
        Wk_p = np.ascontiguousarray(
            np.concatenate([W_embed[j, :, :, k] for j in order], axis=0))
        bk_p = np.tile(
            np.array([b_embed[j, k] for j in order], dtype=np.float32)[None, :],
            (128, 1))
        in_maps.append({"xT": xT, "Wb": Wb_p, "Wk": Wk_p,
                        "bk": np.ascontiguousarray(bk_p)})
    return in_maps, orders


def _assemble(results, orders):
    big = np.empty((J, J, 2, N * N, 2), np.float32)
    for c in range(8):
        i, k = c % 4, c // 4
        Rf = results[c]["out"].reshape(J, 2, N * N)
        for m in range(4):
            big[orders[c][m], i, :, :, k] = Rf[m]
    return big.reshape(-1, 2)


def kernel(x, Wb, W_embed, b_embed):
    import sys
    if "/opt/trn_rl_repo" not in sys.path:
        sys.path.insert(0, "/opt/trn_rl_repo")
    from concourse.bass_utils import run_bass_kernel_spmd

    nc = _build_module()
    in_maps, orders = _in_maps(x, Wb, W_embed, b_embed)
    res = run_bass_kernel_spmd(nc, in_maps, list(range(8)))
    return _assemble(res.results, orders)
